# revision 4
# baseline (speedup 1.0000x reference)
"""AdaptiveTripletLoss kernel for 8 TRN2 NeuronCores.

The reference is a jax.lax.scan over B=16384 samples that mutates the label
vector in place (lab[i] is normalized at step i, after being read).  The scan
flattens exactly: at step i, lab[j] is normalized iff j < i, and lab[i] itself
is always read raw.  So for every sample i (with triplet partners a_i, b_i):

    l1 = n[a_i] if a_i < i else r[a_i]        (r raw, n = (r-MU)/SIGMA)
    l2 = n[b_i] if b_i < i else r[b_i]
    cond_i = |r_i - l1| >= |r_i - l2|
    dp - dn = s_i * (d1_i - d2_i)   with s_i = -1 if cond_i else +1
    d1_i = ||f_i - f_{a_i}||^2,  d2_i = ||f_i - f_{b_i}||^2
    loss_i = relu(s_i * (d1_i - d2_i) + 0.5 * alpha_i)

alpha_i involves only labels; it and s_i are O(B) scalar work done on the
host.  The O(B*D) memory-bound work (3 feature-row streams, squared-distance
reductions) runs on the 8 NeuronCores, batch-sharded 2048 anchors per core.
Each anchor's 3 rows are packed host-side into one [anchor | f_a | f_b] row of
3*D floats so each SBUF tile needs a single contiguous 3 MiB DMA.
"""

import numpy as np

try:
    import concourse.bass as bass
except ImportError:
    import sys

    sys.path.insert(0, "/opt/trn_rl_repo")
    import concourse.bass as bass

import concourse.mybir as mybir
from concourse.tile import TileContext
from concourse.bass_utils import run_bass_kernel_spmd

B, D = 16384, 2048
NCORES = 8
SHARD = B // NCORES  # 2048 anchors per core
P = 128              # SBUF partitions
NT = SHARD // P      # 16 tiles per core
MU = np.float32(136.72353790613718)
SIGMA = np.float32(62.34640414043511)

_nc_cache = None


def _split_inline_waits(nc, max_inline=1):
    """The staged walrus build rejects compute instructions carrying more than
    one inline sync wait ("Too many sync wait commands").  Hoist excess waits
    onto standalone EventSemaphore instructions on the same engine, which is
    semantically identical (engine program order)."""
    cnt = 0
    for blk in nc.m.functions[0].blocks:
        new_insts = []
        for inst in blk.instructions:
            si = inst.sync_info
            if si is not None and len(si.on_wait) > max_inline:
                waits = list(si.on_wait)
                keep, hoist = waits[:max_inline], waits[max_inline:]
                for w in hoist:
                    cnt += 1
                    new_insts.append(
                        mybir.InstEventSemaphore(
                            name=f"{inst.name}-hoistw{cnt}",
                            engine=inst.engine,
                            sync_info=mybir.SyncInfo(on_wait=[w], on_update=[]),
                        )
                    )
                inst.sync_info = mybir.SyncInfo(
                    on_wait=keep, on_update=list(si.on_update)
                )
            new_insts.append(inst)
        blk.instructions = new_insts
    return nc


def build_nc():
    """Per-core graph: for each of NT anchor tiles, one DMA brings the packed
    [A | Ga | Gb] block [128, 3D]; DVE computes A-Ga and A-Gb; ACT squares
    with accumulate into per-anchor d1/d2 columns; a final batched step forms
    relu(s*(d1-d2)+c)."""
    nc = bass.Bass(trn_type="TRN2")
    f32 = mybir.dt.float32
    bundle = nc.declare_dram_parameter("bundle", [SHARD, 3 * D], f32, isOutput=False)
    svec = nc.declare_dram_parameter("svec", [P, NT], f32, isOutput=False)
    cvec = nc.declare_dram_parameter("cvec", [P, NT], f32, isOutput=False)
    out = nc.declare_dram_parameter("out", [P, NT], f32, isOutput=True)

    b_r = bundle.rearrange("(t p) d -> t p d", p=P)  # [NT, P, 3D]

    SQ = mybir.ActivationFunctionType.Square
    with TileContext(nc) as tc:
        with (
            tc.tile_pool(name="io", bufs=3) as iop,
            tc.tile_pool(name="diff", bufs=3) as dfp,
            tc.tile_pool(name="sq", bufs=2, space="PSUM") as qp,
            tc.tile_pool(name="const", bufs=1) as cp,
        ):
            s_t = cp.tile([P, NT], f32, tag="s")
            c_t = cp.tile([P, NT], f32, tag="c")
            d1_all = cp.tile([P, NT], f32, tag="d1")
            d2_all = cp.tile([P, NT], f32, tag="d2")
            nc.sync.dma_start(out=s_t[:], in_=svec[:])
            nc.sync.dma_start(out=c_t[:], in_=cvec[:])
            for t in range(NT):
                T = iop.tile([P, 3 * D], f32, tag="T")
                nc.sync.dma_start(out=T[:], in_=b_r[t])
                da = dfp.tile([P, D], f32, tag="da")
                db = dfp.tile([P, D], f32, tag="db")
                nc.vector.tensor_sub(da[:], T[:, 0:D], T[:, D : 2 * D])
                nc.vector.tensor_sub(db[:], T[:, 0:D], T[:, 2 * D : 3 * D])
                sqa = qp.tile([P, D], f32, tag="sq")
                sqb = qp.tile([P, D], f32, tag="sq")
                nc.scalar.activation(sqa[:], da[:], SQ, accum_out=d1_all[:, t : t + 1])
                nc.scalar.activation(sqb[:], db[:], SQ, accum_out=d2_all[:, t : t + 1])
            dd = cp.tile([P, NT], f32, tag="dd")
            pre = cp.tile([P, NT], f32, tag="pre")
            loss_t = cp.tile([P, NT], f32, tag="loss")
            nc.vector.tensor_sub(dd[:], d1_all[:], d2_all[:])
            nc.vector.tensor_mul(pre[:], dd[:], s_t[:])
            nc.vector.tensor_add(pre[:], pre[:], c_t[:])
            nc.vector.tensor_scalar_max(loss_t[:], pre[:], 0.0)
            nc.sync.dma_start(out=out[:], in_=loss_t[:])
    return _split_inline_waits(nc)


def _host_prep(minibatch_features, label, idx1, idx2):
    feats = np.ascontiguousarray(np.asarray(minibatch_features, dtype=np.float32))
    r = np.asarray(label, dtype=np.float32)
    i1 = np.asarray(idx1).astype(np.int64)
    i2 = np.asarray(idx2).astype(np.int64)
    i = np.arange(B, dtype=np.int64)
    a = (i + 1 + i1 % (B - 1)) % B
    b = (i + 1 + i2 % (B - 1)) % B
    b = np.where(b == a, (i + 1 + (i2 + 1) % (B - 1)) % B, b)

    n = ((r - MU) / SIGMA).astype(np.float32)
    l1 = np.where(a < i, n[a], r[a]).astype(np.float32)
    l2 = np.where(b < i, n[b], r[b]).astype(np.float32)
    cond = np.abs(r - l1) >= np.abs(r - l2)
    near_l = np.where(cond, l2, l1)
    far_l = np.where(cond, l1, l2)
    nl = ((near_l - MU) / SIGMA).astype(np.float32)
    fl = ((far_l - MU) / SIGMA).astype(np.float32)
    alpha = ((n - fl) * (n - fl) - (n - nl) * (n - nl)).astype(np.float32)
    c = (np.float32(0.5) * alpha).astype(np.float32)
    s = np.where(cond, np.float32(-1.0), np.float32(1.0)).astype(np.float32)

    in_maps = []
    for ci in range(NCORES):
        sl = slice(ci * SHARD, (ci + 1) * SHARD)
        bund = np.empty((SHARD, 3 * D), dtype=np.float32)
        bund[:, 0:D] = feats[sl]
        bund[:, D : 2 * D] = feats[a[sl]]
        bund[:, 2 * D : 3 * D] = feats[b[sl]]
        in_maps.append(
            {
                "bundle": bund,
                "svec": np.ascontiguousarray(s[sl].reshape(NT, P).T),
                "cvec": np.ascontiguousarray(c[sl].reshape(NT, P).T),
            }
        )
    return in_maps


def _run_device(in_maps, trace=False, **kwargs):
    global _nc_cache
    if _nc_cache is None:
        _nc_cache = build_nc()
    return run_bass_kernel_spmd(
        _nc_cache, in_maps, core_ids=list(range(NCORES)), trace=trace, **kwargs
    )


def kernel(minibatch_features, label, idx1, idx2):
    in_maps = _host_prep(minibatch_features, label, idx1, idx2)
    res = _run_device(in_maps)
    total = np.float64(0.0)
    for ci in range(NCORES):
        total += np.asarray(res.results[ci]["out"], dtype=np.float64).sum()
    return np.asarray(total, dtype=np.float32)


# revision 6
# speedup vs baseline: 1.1849x; 1.1849x over previous
"""AdaptiveTripletLoss kernel for 8 TRN2 NeuronCores.

The reference is a jax.lax.scan over B=16384 samples that mutates the label
vector in place (lab[i] is normalized at step i, after being read).  The scan
flattens exactly: at step i, lab[j] is normalized iff j < i, and lab[i] itself
is always read raw.  So for every sample i (with triplet partners a_i, b_i):

    l1 = n[a_i] if a_i < i else r[a_i]        (r raw, n = (r-MU)/SIGMA)
    l2 = n[b_i] if b_i < i else r[b_i]
    cond_i = |r_i - l1| >= |r_i - l2|
    dp - dn = s_i * (d1_i - d2_i)   with s_i = -1 if cond_i else +1
    d1_i = ||f_i - f_{a_i}||^2,  d2_i = ||f_i - f_{b_i}||^2
    loss_i = relu(s_i * (d1_i - d2_i) + 0.5 * alpha_i)

alpha_i involves only labels; it and s_i are O(B) scalar work done on the
host.  The O(B*D) memory-bound work (3 feature-row streams, squared-distance
reductions) runs on the 8 NeuronCores, batch-sharded 2048 anchors per core.
Each anchor's rows are packed host-side into one [anchor | f_a | f_b] row of
3*D floats so each 128-anchor tile is a single contiguous 3 MiB DMA with
24 KiB per-row packets (peak per-packet DMA efficiency).

Device program per core (raw Bass, manual semaphores):
  SP   : one 3 MiB DMA per tile (16 tiles); s/c table early; out last
  DVE  : da = A - Ga, db = A - Gb per tile; finale relu(s*(d1-d2)+c)
  ACT  : square-accumulate da/db -> d1a/d2a columns (in-place squares)

DMA completion ordering: completions of concurrently-issued DMAs are not
ordered, so each bundle DMA gets a lane semaphore (t % NBUF); a lane's next
DMA is issue-gated on the consumer having retired that lane's previous tile,
making per-lane cumulative counts race-free (the same scheme Tile uses with
its DMAHW lanes).  Semaphores are cleared at the END of the program (after
the Block-exit barrier), leaving them zero for re-execution of this NEFF.
"""

import contextlib

import numpy as np

try:
    import concourse.bass as bass
except ImportError:
    import sys

    sys.path.insert(0, "/opt/trn_rl_repo")
    import concourse.bass as bass

import concourse.mybir as mybir
from concourse.bass_utils import run_bass_kernel_spmd

B, D = 16384, 2048
NCORES = 8
SHARD = B // NCORES  # 2048 anchors per core
P = 128              # SBUF partitions
NT = SHARD // P      # 16 tiles per core
MU = np.float32(136.72353790613718)
SIGMA = np.float32(62.34640414043511)

F32 = mybir.dt.float32
SQ = mybir.ActivationFunctionType.Square
NBUF = 6   # bundle tile slots == DMA lanes
NDB = 3    # da/db slots

_nc_cache = None


def _split_inline_waits(nc, max_inline=1):
    """The staged walrus build rejects compute instructions carrying more than
    one inline sync wait ("Too many sync wait commands").  Hoist excess waits
    onto standalone EventSemaphore instructions on the same engine, which is
    semantically identical (engine program order)."""
    cnt = 0
    for blk in nc.m.functions[0].blocks:
        new_insts = []
        for inst in blk.instructions:
            si = inst.sync_info
            if si is not None and len(si.on_wait) > max_inline:
                waits = list(si.on_wait)
                keep, hoist = waits[:max_inline], waits[max_inline:]
                for w in hoist:
                    cnt += 1
                    new_insts.append(
                        mybir.InstEventSemaphore(
                            name=f"{inst.name}-hoistw{cnt}",
                            engine=inst.engine,
                            sync_info=mybir.SyncInfo(on_wait=[w], on_update=[]),
                        )
                    )
                inst.sync_info = mybir.SyncInfo(
                    on_wait=keep, on_update=list(si.on_update)
                )
            new_insts.append(inst)
        blk.instructions = new_insts
    return nc


def build_nc():
    nc = bass.Bass(trn_type="TRN2")
    bundle = nc.declare_dram_parameter("bundle", [SHARD, 3 * D], F32, isOutput=False)
    sc = nc.declare_dram_parameter("sc", [P, 2 * NT], F32, isOutput=False)
    out = nc.declare_dram_parameter("out", [P, NT], F32, isOutput=True)

    b_r = bundle.rearrange("(t p) d -> t p d", p=P)  # [NT, P, 3D]

    es = contextlib.ExitStack()
    with es:
        T = [
            es.enter_context(nc.sbuf_tensor(f"Tb{i}", [P, 3 * D], F32))
            for i in range(NBUF)
        ]
        da = [
            es.enter_context(nc.sbuf_tensor(f"da{i}", [P, D], F32))
            for i in range(NDB)
        ]
        db = [
            es.enter_context(nc.sbuf_tensor(f"db{i}", [P, D], F32))
            for i in range(NDB)
        ]
        sct = es.enter_context(nc.sbuf_tensor("sct", [P, 2 * NT], F32))
        d1a = es.enter_context(nc.sbuf_tensor("d1a", [P, NT], F32))
        d2a = es.enter_context(nc.sbuf_tensor("d2a", [P, NT], F32))
        dd = es.enter_context(nc.sbuf_tensor("dd", [P, NT], F32))
        pre = es.enter_context(nc.sbuf_tensor("pre", [P, NT], F32))
        loss = es.enter_context(nc.sbuf_tensor("loss", [P, NT], F32))
        zb = es.enter_context(nc.sbuf_tensor("zb", [P, 1], F32))  # ACT zero bias

        dlane = [es.enter_context(nc.semaphore(f"dl{i}")) for i in range(NBUF)]
        scsem = es.enter_context(nc.semaphore("scsem"))
        osem = es.enter_context(nc.semaphore("osem"))
        vsem = es.enter_context(nc.semaphore("vsem"))
        asem = es.enter_context(nc.semaphore("asem"))
        gsem = es.enter_context(nc.semaphore("gsem"))
        all_sems = dlane + [scsem, osem, vsem, asem, gsem]

        s_ap = sct[:, 0:NT]
        c_ap = sct[:, NT : 2 * NT]

        with nc.Block() as block:

            @block.gpsimd
            def _(gpsimd):
                gpsimd.memset(zb[:], 0.0).then_inc(gsem, 1)

            @block.sync
            def _(sync):
                for t in range(NT):
                    lane = dlane[t % NBUF]
                    if t >= NBUF:
                        # T slot free once both subs of tile t-NBUF retired;
                        # also orders this lane's previous completion.
                        sync.wait_ge(vsem, 2 * (t - NBUF) + 2)
                        sync.wait_ge(lane, 16 * (t // NBUF))
                    sync.dma_start(out=T[t % NBUF][:], in_=b_r[t]).then_inc(lane, 16)
                    if t == 0:
                        sync.dma_start(out=sct[:], in_=sc[:]).then_inc(scsem, 16)
                # final out DMA after DVE finale (32 subs + 4 finale ops)
                sync.wait_ge(vsem, 2 * NT + 4)
                sync.dma_start(out=out[:], in_=loss[:]).then_inc(osem, 16)
                sync.wait_ge(osem, 16)

            @block.vector
            def _(vector):
                for t in range(NT):
                    vector.wait_ge(dlane[t % NBUF], 16 * (t // NBUF + 1))
                    if t >= NDB:
                        # da/db slot free once ACT consumed tile t-NDB
                        vector.wait_ge(asem, 2 * (t - NDB) + 2)
                    nc.vector.tensor_sub(
                        da[t % NDB][:], T[t % NBUF][:, 0:D], T[t % NBUF][:, D : 2 * D]
                    ).then_inc(vsem, 1)
                    nc.vector.tensor_sub(
                        db[t % NDB][:], T[t % NBUF][:, 0:D], T[t % NBUF][:, 2 * D :]
                    ).then_inc(vsem, 1)
                vector.wait_ge(asem, 2 * NT)
                vector.wait_ge(scsem, 16)
                # Self-waits: engine pipelining does not order same-engine
                # dependent ops; each op must see the previous one retire.
                nc.vector.tensor_sub(dd[:], d1a[:], d2a[:]).then_inc(vsem, 1)
                vector.wait_ge(vsem, 2 * NT + 1)
                nc.vector.tensor_mul(pre[:], dd[:], s_ap).then_inc(vsem, 1)
                vector.wait_ge(vsem, 2 * NT + 2)
                nc.vector.tensor_add(pre[:], pre[:], c_ap).then_inc(vsem, 1)
                vector.wait_ge(vsem, 2 * NT + 3)
                nc.vector.tensor_scalar_max(loss[:], pre[:], 0.0).then_inc(vsem, 1)

            @block.scalar
            def _(scalar):
                scalar.wait_ge(gsem, 1)
                for t in range(NT):
                    # Square in place: the squared tile is never consumed, only
                    # the accumulator; in-place avoids a shared scratch whose
                    # reuse across ACT instructions would be a WAW hazard.
                    scalar.wait_ge(vsem, 2 * t + 1)
                    nc.scalar.activation(
                        da[t % NDB][:], da[t % NDB][:], SQ,
                        bias=zb[:], accum_out=d1a[:, t : t + 1],
                    ).then_inc(asem, 1)
                    scalar.wait_ge(vsem, 2 * t + 2)
                    nc.scalar.activation(
                        db[t % NDB][:], db[t % NDB][:], SQ,
                        bias=zb[:], accum_out=d2a[:, t : t + 1],
                    ).then_inc(asem, 1)

        # After the Block-exit all-engine barrier: leave the sems cleared for
        # any re-execution of this NEFF.
        for s in all_sems:
            nc.sync.sem_clear(s)

    return _split_inline_waits(nc)


def _host_scalars(label, idx1, idx2):
    """Flattened-scan label math: triplet indices (a, b), sign s = -1 if
    cond else +1, and c = 0.5*alpha, exactly as the reference computes them
    (f32 ops in the same order)."""
    r = np.asarray(label, dtype=np.float32)
    i1 = np.asarray(idx1).astype(np.int64)
    i2 = np.asarray(idx2).astype(np.int64)
    i = np.arange(B, dtype=np.int64)
    a = (i + 1 + i1 % (B - 1)) % B
    b = (i + 1 + i2 % (B - 1)) % B
    b = np.where(b == a, (i + 1 + (i2 + 1) % (B - 1)) % B, b)

    n = ((r - MU) / SIGMA).astype(np.float32)
    l1 = np.where(a < i, n[a], r[a]).astype(np.float32)
    l2 = np.where(b < i, n[b], r[b]).astype(np.float32)
    cond = np.abs(r - l1) >= np.abs(r - l2)
    near_l = np.where(cond, l2, l1)
    far_l = np.where(cond, l1, l2)
    nl = ((near_l - MU) / SIGMA).astype(np.float32)
    fl = ((far_l - MU) / SIGMA).astype(np.float32)
    alpha = ((n - fl) * (n - fl) - (n - nl) * (n - nl)).astype(np.float32)
    c = (np.float32(0.5) * alpha).astype(np.float32)
    s = np.where(cond, np.float32(-1.0), np.float32(1.0)).astype(np.float32)
    return a, b, s, c


def _host_prep(minibatch_features, label, idx1, idx2):
    feats = np.ascontiguousarray(np.asarray(minibatch_features, dtype=np.float32))
    a, b, s, c = _host_scalars(label, idx1, idx2)

    in_maps = []
    for ci in range(NCORES):
        sl = slice(ci * SHARD, (ci + 1) * SHARD)
        bund = np.empty((SHARD, 3 * D), dtype=np.float32)
        bund[:, 0:D] = feats[sl]
        bund[:, D : 2 * D] = feats[a[sl]]
        bund[:, 2 * D : 3 * D] = feats[b[sl]]
        scm = np.empty((P, 2 * NT), dtype=np.float32)
        scm[:, :NT] = s[sl].reshape(NT, P).T
        scm[:, NT:] = c[sl].reshape(NT, P).T
        in_maps.append({"bundle": bund, "sc": scm})
    return in_maps


def _run_device(in_maps, trace=False, **kwargs):
    global _nc_cache
    if _nc_cache is None:
        _nc_cache = build_nc()
    return run_bass_kernel_spmd(
        _nc_cache, in_maps, core_ids=list(range(NCORES)), trace=trace, **kwargs
    )


def kernel(minibatch_features, label, idx1, idx2):
    in_maps = _host_prep(minibatch_features, label, idx1, idx2)
    res = _run_device(in_maps)
    total = np.float64(0.0)
    for ci in range(NCORES):
        total += np.asarray(res.results[ci]["out"], dtype=np.float64).sum()
    return np.asarray(total, dtype=np.float32)


# revision 9
# speedup vs baseline: 1.2172x; 1.0272x over previous
"""AdaptiveTripletLoss kernel for 8 TRN2 NeuronCores.

The reference is a jax.lax.scan over B=16384 samples that mutates the label
vector in place (lab[i] is normalized at step i, after being read).  The scan
flattens exactly: at step i, lab[j] is normalized iff j < i, and lab[i] itself
is always read raw.  So for every sample i (with triplet partners a_i, b_i):

    l1 = n[a_i] if a_i < i else r[a_i]        (r raw, n = (r-MU)/SIGMA)
    l2 = n[b_i] if b_i < i else r[b_i]
    cond_i = |r_i - l1| >= |r_i - l2|
    dp - dn = s_i * (d1_i - d2_i)   with s_i = -1 if cond_i else +1
    d1_i = ||f_i - f_{a_i}||^2,  d2_i = ||f_i - f_{b_i}||^2
    loss_i = relu(s_i * (d1_i - d2_i) + 0.5 * alpha_i)

alpha_i involves only labels; it and s_i are O(B) scalar work done on the
host.  The O(B*D) memory-bound work (3 feature-row streams, squared-distance
reductions) runs on the 8 NeuronCores, batch-sharded 2048 anchors per core.
Each anchor's rows are packed host-side into one [anchor | f_a | f_b] row of
3*D floats so each 128-anchor tile is a single contiguous 3 MiB DMA with
24 KiB per-row packets (peak per-packet DMA efficiency).

Device program per core (raw Bass, manual semaphores):
  SP   : one 3 MiB DMA per tile (16 tiles); s/c table early; out last
  DVE  : da = A - Ga, db = A - Gb per tile; finale relu(s*(d1-d2)+c)
  ACT  : square-accumulate da/db -> d1a/d2a columns (in-place squares)

DMA completion ordering: completions of concurrently-issued DMAs are not
ordered, so each bundle DMA gets a lane semaphore (t % NBUF); a lane's next
DMA is issue-gated on the consumer having retired that lane's previous tile,
making per-lane cumulative counts race-free (the same scheme Tile uses with
its DMAHW lanes).  Semaphores are cleared at the END of the program (after
the Block-exit barrier), leaving them zero for re-execution of this NEFF.
"""

import contextlib

import numpy as np

try:
    import concourse.bass as bass
except ImportError:
    import sys

    sys.path.insert(0, "/opt/trn_rl_repo")
    import concourse.bass as bass

import concourse.mybir as mybir
from concourse.bass_utils import run_bass_kernel_spmd

B, D = 16384, 2048
NCORES = 8
SHARD = B // NCORES  # 2048 anchors per core
P = 128              # SBUF partitions
NT = SHARD // P      # 16 tiles per core
MU = np.float32(136.72353790613718)
SIGMA = np.float32(62.34640414043511)

F32 = mybir.dt.float32
SQ = mybir.ActivationFunctionType.Square
NBUF = 6   # bundle tile slots == DMA lanes
NDB = 3    # da/db slots

_nc_cache = None


def _split_inline_waits(nc, max_inline=1):
    """The staged walrus build rejects compute instructions carrying more than
    one inline sync wait ("Too many sync wait commands").  Hoist excess waits
    onto standalone EventSemaphore instructions on the same engine, which is
    semantically identical (engine program order)."""
    cnt = 0
    for blk in nc.m.functions[0].blocks:
        new_insts = []
        for inst in blk.instructions:
            si = inst.sync_info
            if si is not None and len(si.on_wait) > max_inline:
                waits = list(si.on_wait)
                keep, hoist = waits[:max_inline], waits[max_inline:]
                for w in hoist:
                    cnt += 1
                    new_insts.append(
                        mybir.InstEventSemaphore(
                            name=f"{inst.name}-hoistw{cnt}",
                            engine=inst.engine,
                            sync_info=mybir.SyncInfo(on_wait=[w], on_update=[]),
                        )
                    )
                inst.sync_info = mybir.SyncInfo(
                    on_wait=keep, on_update=list(si.on_update)
                )
            new_insts.append(inst)
        blk.instructions = new_insts
    return nc


def _strip_init_barrier(nc):
    """Bass.__init__ unconditionally memsets 4 const APs and runs an
    all-engine barrier (~3.3us on HW) before the kernel body.  This kernel
    reads none of those const APs, so drop the memsets and that first barrier
    (everything up to the first branch in the entry block)."""
    blk = nc.m.functions[0].blocks[0]
    kept = []
    seen_branch = False
    for inst in blk.instructions:
        if isinstance(inst, mybir.InstUnconditionalBranch):
            seen_branch = True
        if not seen_branch:
            # Only const-AP memsets and the init barrier live before the
            # first branch; the kernel's own memset is inside a body block.
            if isinstance(
                inst, (mybir.InstMemset, mybir.InstDrain, mybir.InstEventSemaphore)
            ):
                continue
        kept.append(inst)
    blk.instructions = kept
    return nc


def build_nc():
    nc = bass.Bass(trn_type="TRN2")
    bundle = nc.declare_dram_parameter("bundle", [SHARD, 3 * D], F32, isOutput=False)
    sc = nc.declare_dram_parameter("sc", [P, 2 * NT], F32, isOutput=False)
    out = nc.declare_dram_parameter("out", [P, NT], F32, isOutput=True)

    b_r = bundle.rearrange("(t p) d -> t p d", p=P)  # [NT, P, 3D]

    es = contextlib.ExitStack()
    with es:
        T = [
            es.enter_context(nc.sbuf_tensor(f"Tb{i}", [P, 3 * D], F32))
            for i in range(NBUF)
        ]
        da = [
            es.enter_context(nc.sbuf_tensor(f"da{i}", [P, D], F32))
            for i in range(NDB)
        ]
        db = [
            es.enter_context(nc.sbuf_tensor(f"db{i}", [P, D], F32))
            for i in range(NDB)
        ]
        sct = es.enter_context(nc.sbuf_tensor("sct", [P, 2 * NT], F32))
        d1a = es.enter_context(nc.sbuf_tensor("d1a", [P, NT], F32))
        d2a = es.enter_context(nc.sbuf_tensor("d2a", [P, NT], F32))
        dd = es.enter_context(nc.sbuf_tensor("dd", [P, NT], F32))
        pre = es.enter_context(nc.sbuf_tensor("pre", [P, NT], F32))
        loss = es.enter_context(nc.sbuf_tensor("loss", [P, NT], F32))
        zb = es.enter_context(nc.sbuf_tensor("zb", [P, 1], F32))  # ACT zero bias

        dlane = [es.enter_context(nc.semaphore(f"dl{i}")) for i in range(NBUF)]
        scsem = es.enter_context(nc.semaphore("scsem"))
        osem = es.enter_context(nc.semaphore("osem"))
        vsem = es.enter_context(nc.semaphore("vsem"))
        asem = es.enter_context(nc.semaphore("asem"))
        gsem = es.enter_context(nc.semaphore("gsem"))
        all_sems = dlane + [scsem, osem, vsem, asem, gsem]

        s_ap = sct[:, 0:NT]
        c_ap = sct[:, NT : 2 * NT]

        with nc.Block() as block:

            @block.gpsimd
            def _(gpsimd):
                gpsimd.memset(zb[:], 0.0).then_inc(gsem, 1)

            @block.sync
            def _(sync):
                for t in range(NT):
                    lane = dlane[t % NBUF]
                    if t >= NBUF:
                        # T slot free once both subs of tile t-NBUF retired;
                        # also orders this lane's previous completion.
                        sync.wait_ge(vsem, 2 * (t - NBUF) + 2)
                        sync.wait_ge(lane, 16 * (t // NBUF))
                    sync.dma_start(out=T[t % NBUF][:], in_=b_r[t]).then_inc(lane, 16)
                    if t == 0:
                        sync.dma_start(out=sct[:], in_=sc[:]).then_inc(scsem, 16)
                # final out DMA after DVE finale (32 subs + 4 finale ops)
                sync.wait_ge(vsem, 2 * NT + 4)
                sync.dma_start(out=out[:], in_=loss[:]).then_inc(osem, 16)
                sync.wait_ge(osem, 16)

            @block.vector
            def _(vector):
                for t in range(NT):
                    vector.wait_ge(dlane[t % NBUF], 16 * (t // NBUF + 1))
                    if t >= NDB:
                        # da/db slot free once ACT consumed tile t-NDB
                        vector.wait_ge(asem, 2 * (t - NDB) + 2)
                    nc.vector.tensor_sub(
                        da[t % NDB][:], T[t % NBUF][:, 0:D], T[t % NBUF][:, D : 2 * D]
                    ).then_inc(vsem, 1)
                    nc.vector.tensor_sub(
                        db[t % NDB][:], T[t % NBUF][:, 0:D], T[t % NBUF][:, 2 * D :]
                    ).then_inc(vsem, 1)
                vector.wait_ge(asem, 2 * NT)
                vector.wait_ge(scsem, 16)
                # Self-waits: engine pipelining does not order same-engine
                # dependent ops; each op must see the previous one retire.
                nc.vector.tensor_sub(dd[:], d1a[:], d2a[:]).then_inc(vsem, 1)
                vector.wait_ge(vsem, 2 * NT + 1)
                nc.vector.tensor_mul(pre[:], dd[:], s_ap).then_inc(vsem, 1)
                vector.wait_ge(vsem, 2 * NT + 2)
                nc.vector.tensor_add(pre[:], pre[:], c_ap).then_inc(vsem, 1)
                vector.wait_ge(vsem, 2 * NT + 3)
                nc.vector.tensor_scalar_max(loss[:], pre[:], 0.0).then_inc(vsem, 1)

            @block.scalar
            def _(scalar):
                scalar.wait_ge(gsem, 1)
                for t in range(NT):
                    # Square in place: the squared tile is never consumed, only
                    # the accumulator; in-place avoids a shared scratch whose
                    # reuse across ACT instructions would be a WAW hazard.
                    scalar.wait_ge(vsem, 2 * t + 1)
                    nc.scalar.activation(
                        da[t % NDB][:], da[t % NDB][:], SQ,
                        bias=zb[:], accum_out=d1a[:, t : t + 1],
                    ).then_inc(asem, 1)
                    scalar.wait_ge(vsem, 2 * t + 2)
                    nc.scalar.activation(
                        db[t % NDB][:], db[t % NDB][:], SQ,
                        bias=zb[:], accum_out=d2a[:, t : t + 1],
                    ).then_inc(asem, 1)

        # After the Block-exit all-engine barrier: leave the sems cleared for
        # any re-execution of this NEFF.
        for s in all_sems:
            nc.sync.sem_clear(s)

    return _split_inline_waits(_strip_init_barrier(nc))


def _host_scalars(label, idx1, idx2):
    """Flattened-scan label math: triplet indices (a, b), sign s = -1 if
    cond else +1, and c = 0.5*alpha, exactly as the reference computes them
    (f32 ops in the same order)."""
    r = np.asarray(label, dtype=np.float32)
    i1 = np.asarray(idx1).astype(np.int64)
    i2 = np.asarray(idx2).astype(np.int64)
    i = np.arange(B, dtype=np.int64)
    a = (i + 1 + i1 % (B - 1)) % B
    b = (i + 1 + i2 % (B - 1)) % B
    b = np.where(b == a, (i + 1 + (i2 + 1) % (B - 1)) % B, b)

    n = ((r - MU) / SIGMA).astype(np.float32)
    l1 = np.where(a < i, n[a], r[a]).astype(np.float32)
    l2 = np.where(b < i, n[b], r[b]).astype(np.float32)
    cond = np.abs(r - l1) >= np.abs(r - l2)
    near_l = np.where(cond, l2, l1)
    far_l = np.where(cond, l1, l2)
    nl = ((near_l - MU) / SIGMA).astype(np.float32)
    fl = ((far_l - MU) / SIGMA).astype(np.float32)
    alpha = ((n - fl) * (n - fl) - (n - nl) * (n - nl)).astype(np.float32)
    c = (np.float32(0.5) * alpha).astype(np.float32)
    s = np.where(cond, np.float32(-1.0), np.float32(1.0)).astype(np.float32)
    return a, b, s, c


def _host_prep(minibatch_features, label, idx1, idx2):
    feats = np.ascontiguousarray(np.asarray(minibatch_features, dtype=np.float32))
    a, b, s, c = _host_scalars(label, idx1, idx2)

    in_maps = []
    for ci in range(NCORES):
        sl = slice(ci * SHARD, (ci + 1) * SHARD)
        bund = np.empty((SHARD, 3 * D), dtype=np.float32)
        bund[:, 0:D] = feats[sl]
        bund[:, D : 2 * D] = feats[a[sl]]
        bund[:, 2 * D : 3 * D] = feats[b[sl]]
        scm = np.empty((P, 2 * NT), dtype=np.float32)
        scm[:, :NT] = s[sl].reshape(NT, P).T
        scm[:, NT:] = c[sl].reshape(NT, P).T
        in_maps.append({"bundle": bund, "sc": scm})
    return in_maps


def _run_device(in_maps, trace=False, **kwargs):
    global _nc_cache
    if _nc_cache is None:
        _nc_cache = build_nc()
    return run_bass_kernel_spmd(
        _nc_cache, in_maps, core_ids=list(range(NCORES)), trace=trace, **kwargs
    )


def kernel(minibatch_features, label, idx1, idx2):
    in_maps = _host_prep(minibatch_features, label, idx1, idx2)
    res = _run_device(in_maps)
    total = np.float64(0.0)
    for ci in range(NCORES):
        total += np.asarray(res.results[ci]["out"], dtype=np.float64).sum()
    return np.asarray(total, dtype=np.float32)


# revision 12
# speedup vs baseline: 1.5057x; 1.2371x over previous
"""AdaptiveTripletLoss kernel for 8 TRN2 NeuronCores.

The reference is a jax.lax.scan over B=16384 samples that mutates the label
vector in place (lab[i] is normalized at step i, after being read).  The scan
flattens exactly: at step i, lab[j] is normalized iff j < i, and lab[i] itself
is always read raw.  So for every sample i (with triplet partners a_i, b_i):

    l1 = n[a_i] if a_i < i else r[a_i]        (r raw, n = (r-MU)/SIGMA)
    l2 = n[b_i] if b_i < i else r[b_i]
    cond_i = |r_i - l1| >= |r_i - l2|
    dp - dn = s_i * (d1_i - d2_i)   with s_i = -1 if cond_i else +1
    d1_i = ||f_i - f_{a_i}||^2,  d2_i = ||f_i - f_{b_i}||^2
    loss_i = relu(s_i * (d1_i - d2_i) + 0.5 * alpha_i)

alpha_i involves only labels; it and s_i are O(B) scalar work done on the
host.  The O(B*D) memory-bound work (3 feature-row streams, squared-distance
reductions) runs on the 8 NeuronCores, batch-sharded 2048 anchors per core.

Features stream in bfloat16 (end-to-end loss error ~1e-4, far inside the 2e-2
gate), halving DMA traffic vs f32.  Rows are packed host-side as
[anchor|f_a|f_b] bundles with TWO anchors per DRAM row ("supertile") so
per-row packets stay at 24 KiB — the size at which the SDMA engines run at
line rate (12 KiB packets measured ~25% slower per byte).

Device program per core (raw Bass, manual semaphores):
  SP   : one 3 MiB DMA per supertile (8 supertiles of 2x128 anchors)
  DVE  : 4 bf16 subs per supertile into da/db; d2 via fused
         tensor_tensor_reduce (square+sum) on db; finale relu(s*(d1-d2)+c)
  ACT  : square-accumulate da -> d1a columns (in-place squares)

DMA completion ordering: completions of concurrently-issued DMAs are not
ordered, so each DMA gets a lane semaphore (u % NBUF); a lane's next DMA is
issue-gated on the consumer having retired that lane's previous supertile,
making per-lane cumulative counts race-free.  Same-engine dependent ops carry
self-waits (engine pipelining does not order them).  Semaphores are cleared
at the END of the program, leaving them zero for re-execution of this NEFF.
"""

import contextlib

import numpy as np
import ml_dtypes

try:
    import concourse.bass as bass
except ImportError:
    import sys

    sys.path.insert(0, "/opt/trn_rl_repo")
    import concourse.bass as bass

import concourse.mybir as mybir
from concourse.bass_utils import run_bass_kernel_spmd

B, D = 16384, 2048
NCORES = 8
SHARD = B // NCORES  # 2048 anchors per core
P = 128              # SBUF partitions
NT = SHARD // P      # 16 logical tiles per core
NST = NT // 2        # 8 supertiles (2 tiles per DRAM row)
MU = np.float32(136.72353790613718)
SIGMA = np.float32(62.34640414043511)

F32 = mybir.dt.float32
BF16 = mybir.dt.bfloat16
SQ = mybir.ActivationFunctionType.Square
NBUF = 4   # supertile slots == DMA lanes
NDB = 3    # da/db slots

_nc_cache = None


def _split_inline_waits(nc, max_inline=1):
    """The staged walrus build rejects compute instructions carrying more than
    one inline sync wait ("Too many sync wait commands").  Hoist excess waits
    onto standalone EventSemaphore instructions on the same engine, which is
    semantically identical (engine program order)."""
    cnt = 0
    for blk in nc.m.functions[0].blocks:
        new_insts = []
        for inst in blk.instructions:
            si = inst.sync_info
            if si is not None and len(si.on_wait) > max_inline:
                waits = list(si.on_wait)
                keep, hoist = waits[:max_inline], waits[max_inline:]
                for w in hoist:
                    cnt += 1
                    new_insts.append(
                        mybir.InstEventSemaphore(
                            name=f"{inst.name}-hoistw{cnt}",
                            engine=inst.engine,
                            sync_info=mybir.SyncInfo(on_wait=[w], on_update=[]),
                        )
                    )
                inst.sync_info = mybir.SyncInfo(
                    on_wait=keep, on_update=list(si.on_update)
                )
            new_insts.append(inst)
        blk.instructions = new_insts
    return nc


def _strip_init_barrier(nc):
    """Bass.__init__ unconditionally memsets 4 const APs and runs an
    all-engine barrier (~3.3us on HW) before the kernel body.  This kernel
    reads none of those const APs, so drop the memsets and that first barrier
    (everything up to the first branch in the entry block)."""
    blk = nc.m.functions[0].blocks[0]
    kept = []
    seen_branch = False
    for inst in blk.instructions:
        if isinstance(inst, mybir.InstUnconditionalBranch):
            seen_branch = True
        if not seen_branch:
            # Only const-AP memsets and the init barrier live before the
            # first branch; the kernel's own memset is inside a body block.
            if isinstance(
                inst, (mybir.InstMemset, mybir.InstDrain, mybir.InstEventSemaphore)
            ):
                continue
        kept.append(inst)
    blk.instructions = kept
    return nc


def build_nc():
    nc = bass.Bass(trn_type="TRN2")
    # one supertile row = [A|Ga|Gb] of anchor (2u*128+p) ++ same for anchor
    # ((2u+1)*128+p): 12288 bf16 = 24 KiB
    bundle = nc.declare_dram_parameter(
        "bundle", [NST * P, 6 * D], BF16, isOutput=False
    )
    sc = nc.declare_dram_parameter("sc", [P, 2 * NT], F32, isOutput=False)
    out = nc.declare_dram_parameter("out", [P, NT], F32, isOutput=True)

    b_r = bundle.rearrange("(u p) d -> u p d", p=P)  # [NST, P, 6D]

    es = contextlib.ExitStack()
    with es:
        T = [
            es.enter_context(nc.sbuf_tensor(f"Tb{i}", [P, 6 * D], BF16))
            for i in range(NBUF)
        ]
        da = [
            es.enter_context(nc.sbuf_tensor(f"da{i}", [P, 2 * D], BF16))
            for i in range(NDB)
        ]
        db = [
            es.enter_context(nc.sbuf_tensor(f"db{i}", [P, 2 * D], BF16))
            for i in range(NDB)
        ]
        sct = es.enter_context(nc.sbuf_tensor("sct", [P, 2 * NT], F32))
        d1a = es.enter_context(nc.sbuf_tensor("d1a", [P, NT], F32))
        d2a = es.enter_context(nc.sbuf_tensor("d2a", [P, NT], F32))
        dd = es.enter_context(nc.sbuf_tensor("dd", [P, NT], F32))
        pre = es.enter_context(nc.sbuf_tensor("pre", [P, NT], F32))
        loss = es.enter_context(nc.sbuf_tensor("loss", [P, NT], F32))
        zb = es.enter_context(nc.sbuf_tensor("zb", [P, 1], F32))  # ACT zero bias

        dlane = [es.enter_context(nc.semaphore(f"dl{i}")) for i in range(NBUF)]
        scsem = es.enter_context(nc.semaphore("scsem"))
        osem = es.enter_context(nc.semaphore("osem"))
        vsem = es.enter_context(nc.semaphore("vsem"))
        asem = es.enter_context(nc.semaphore("asem"))
        gsem = es.enter_context(nc.semaphore("gsem"))
        all_sems = dlane + [scsem, osem, vsem, asem, gsem]

        s_ap = sct[:, 0:NT]
        c_ap = sct[:, NT : 2 * NT]
        VTOT = 6 * NST  # DVE ops in the supertile loop

        with nc.Block() as block:

            @block.gpsimd
            def _(gpsimd):
                gpsimd.memset(zb[:], 0.0).then_inc(gsem, 1)

            @block.sync
            def _(sync):
                for u in range(NST):
                    lane = dlane[u % NBUF]
                    if u >= NBUF:
                        # T slot free once the 4 subs of supertile u-NBUF
                        # retired; also orders this lane's previous DMA.
                        sync.wait_ge(vsem, 6 * (u - NBUF) + 4)
                        sync.wait_ge(lane, 16 * (u // NBUF))
                    sync.dma_start(out=T[u % NBUF][:], in_=b_r[u]).then_inc(lane, 16)
                    if u == 0:
                        sync.dma_start(out=sct[:], in_=sc[:]).then_inc(scsem, 16)
                sync.wait_ge(vsem, VTOT + 4)
                sync.dma_start(out=out[:], in_=loss[:]).then_inc(osem, 16)
                sync.wait_ge(osem, 16)

            @block.vector
            def _(vector):
                for u in range(NST):
                    A, Dq, Db = T[u % NBUF], da[u % NDB], db[u % NDB]
                    vector.wait_ge(dlane[u % NBUF], 16 * (u // NBUF + 1))
                    if u >= NDB:
                        # da slot free once ACT consumed supertile u-NDB;
                        # db slot free once own ttrs of u-NDB retired.
                        vector.wait_ge(asem, 2 * (u - NDB) + 2)
                        vector.wait_ge(vsem, 6 * (u - NDB) + 6)
                    nc.vector.tensor_sub(
                        Dq[:, 0:D], A[:, 0:D], A[:, D : 2 * D]
                    ).then_inc(vsem, 1)
                    nc.vector.tensor_sub(
                        Db[:, 0:D], A[:, 0:D], A[:, 2 * D : 3 * D]
                    ).then_inc(vsem, 1)
                    nc.vector.tensor_sub(
                        Dq[:, D : 2 * D], A[:, 3 * D : 4 * D], A[:, 4 * D : 5 * D]
                    ).then_inc(vsem, 1)
                    nc.vector.tensor_sub(
                        Db[:, D : 2 * D], A[:, 3 * D : 4 * D], A[:, 5 * D : 6 * D]
                    ).then_inc(vsem, 1)
                    # d2 via fused square+sum on DVE (in-place on db); needs
                    # own subs retired first (same-engine RAW).
                    vector.wait_ge(vsem, 6 * u + 4)
                    nc.vector.scalar_tensor_tensor(
                        Db[:, 0:D], Db[:, 0:D], 1.0, Db[:, 0:D],
                        mybir.AluOpType.bypass, mybir.AluOpType.mult,
                        accum_out=d2a[:, 2 * u : 2 * u + 1],
                    ).then_inc(vsem, 1)
                    nc.vector.scalar_tensor_tensor(
                        Db[:, D : 2 * D], Db[:, D : 2 * D], 1.0, Db[:, D : 2 * D],
                        mybir.AluOpType.bypass, mybir.AluOpType.mult,
                        accum_out=d2a[:, 2 * u + 1 : 2 * u + 2],
                    ).then_inc(vsem, 1)
                vector.wait_ge(asem, 2 * NST)
                vector.wait_ge(scsem, 16)
                # self-wait: dd reads d2a written by this engine's own ttrs
                vector.wait_ge(vsem, VTOT)
                nc.vector.tensor_sub(dd[:], d1a[:], d2a[:]).then_inc(vsem, 1)
                vector.wait_ge(vsem, VTOT + 1)
                nc.vector.tensor_mul(pre[:], dd[:], s_ap).then_inc(vsem, 1)
                vector.wait_ge(vsem, VTOT + 2)
                nc.vector.tensor_add(pre[:], pre[:], c_ap).then_inc(vsem, 1)
                vector.wait_ge(vsem, VTOT + 3)
                nc.vector.tensor_scalar_max(loss[:], pre[:], 0.0).then_inc(vsem, 1)

            @block.scalar
            def _(scalar):
                scalar.wait_ge(gsem, 1)
                for u in range(NST):
                    Dq = da[u % NDB]
                    # Square in place: only the accumulator is consumed.
                    scalar.wait_ge(vsem, 6 * u + 1)
                    nc.scalar.activation(
                        Dq[:, 0:D], Dq[:, 0:D], SQ,
                        bias=zb[:], accum_out=d1a[:, 2 * u : 2 * u + 1],
                    ).then_inc(asem, 1)
                    scalar.wait_ge(vsem, 6 * u + 3)
                    nc.scalar.activation(
                        Dq[:, D : 2 * D], Dq[:, D : 2 * D], SQ,
                        bias=zb[:], accum_out=d1a[:, 2 * u + 1 : 2 * u + 2],
                    ).then_inc(asem, 1)

        # After the Block-exit all-engine barrier: leave the sems cleared for
        # any re-execution of this NEFF.
        for s in all_sems:
            nc.sync.sem_clear(s)

    return _split_inline_waits(_strip_init_barrier(nc))


def _host_scalars(label, idx1, idx2):
    """Flattened-scan label math: triplet indices (a, b), sign s = -1 if
    cond else +1, and c = 0.5*alpha, exactly as the reference computes them
    (f32 ops in the same order)."""
    r = np.asarray(label, dtype=np.float32)
    i1 = np.asarray(idx1).astype(np.int64)
    i2 = np.asarray(idx2).astype(np.int64)
    i = np.arange(B, dtype=np.int64)
    a = (i + 1 + i1 % (B - 1)) % B
    b = (i + 1 + i2 % (B - 1)) % B
    b = np.where(b == a, (i + 1 + (i2 + 1) % (B - 1)) % B, b)

    n = ((r - MU) / SIGMA).astype(np.float32)
    l1 = np.where(a < i, n[a], r[a]).astype(np.float32)
    l2 = np.where(b < i, n[b], r[b]).astype(np.float32)
    cond = np.abs(r - l1) >= np.abs(r - l2)
    near_l = np.where(cond, l2, l1)
    far_l = np.where(cond, l1, l2)
    nl = ((near_l - MU) / SIGMA).astype(np.float32)
    fl = ((far_l - MU) / SIGMA).astype(np.float32)
    alpha = ((n - fl) * (n - fl) - (n - nl) * (n - nl)).astype(np.float32)
    c = (np.float32(0.5) * alpha).astype(np.float32)
    s = np.where(cond, np.float32(-1.0), np.float32(1.0)).astype(np.float32)
    return a, b, s, c


def _host_prep(minibatch_features, label, idx1, idx2):
    feats = np.asarray(minibatch_features, dtype=np.float32)
    a, b, s, c = _host_scalars(label, idx1, idx2)
    fb = feats.astype(ml_dtypes.bfloat16)

    in_maps = []
    for ci in range(NCORES):
        sl = slice(ci * SHARD, (ci + 1) * SHARD)
        flat = np.concatenate([fb[sl], fb[a[sl]], fb[b[sl]]], axis=1)  # [SHARD,3D]
        # pair consecutive tiles into supertile rows: [NST, 2, P, 3D] ->
        # [NST, P, 2, 3D] -> [NST*P, 6D]
        bund = np.ascontiguousarray(
            flat.reshape(NST, 2, P, 3 * D).transpose(0, 2, 1, 3)
        ).reshape(NST * P, 6 * D)
        scm = np.empty((P, 2 * NT), dtype=np.float32)
        scm[:, :NT] = s[sl].reshape(NT, P).T
        scm[:, NT:] = c[sl].reshape(NT, P).T
        in_maps.append({"bundle": bund, "sc": scm})
    return in_maps


def _run_device(in_maps, trace=False, **kwargs):
    global _nc_cache
    if _nc_cache is None:
        _nc_cache = build_nc()
    return run_bass_kernel_spmd(
        _nc_cache, in_maps, core_ids=list(range(NCORES)), trace=trace, **kwargs
    )


def kernel(minibatch_features, label, idx1, idx2):
    in_maps = _host_prep(minibatch_features, label, idx1, idx2)
    res = _run_device(in_maps)
    total = np.float64(0.0)
    for ci in range(NCORES):
        total += np.asarray(res.results[ci]["out"], dtype=np.float64).sum()
    return np.asarray(total, dtype=np.float32)


# revision 16
# speedup vs baseline: 2.0758x; 1.3786x over previous
"""AdaptiveTripletLoss kernel for 8 TRN2 NeuronCores.

The reference is a jax.lax.scan over B=16384 samples that mutates the label
vector in place (lab[i] is normalized at step i, after being read).  The scan
flattens exactly: at step i, lab[j] is normalized iff j < i, and lab[i] itself
is always read raw.  So for every sample i (with triplet partners a_i, b_i):

    l1 = n[a_i] if a_i < i else r[a_i]        (r raw, n = (r-MU)/SIGMA)
    l2 = n[b_i] if b_i < i else r[b_i]
    cond_i = |r_i - l1| >= |r_i - l2|
    dp - dn = s_i * (d1_i - d2_i)   with s_i = -1 if cond_i else +1
    d1_i = ||f_i - f_{a_i}||^2,  d2_i = ||f_i - f_{b_i}||^2
    loss_i = relu(s_i * (d1_i - d2_i) + 0.5 * alpha_i)

alpha_i involves only labels; it and s_i are O(B) scalar work done on the
host.  The O(B*D) memory-bound work (3 feature-row streams, squared-distance
reductions) runs on the 8 NeuronCores, batch-sharded 2048 anchors per core.

Features stream in bfloat16 (end-to-end loss error ~1e-4, far inside the 2e-2
gate), halving DMA traffic vs f32.  Rows are packed host-side as
[anchor|f_a|f_b] bundles with TWO anchors per DRAM row ("supertile") so
per-row packets stay at 24 KiB — the size at which the SDMA engines run at
line rate (12 KiB packets measured ~25% slower per byte).

Device program per core (raw Bass, manual semaphores):
  SP   : one 3 MiB DMA per supertile (8 supertiles of 2x128 anchors)
  DVE  : 4 bf16 subs per supertile into da/db; d2 via fused
         tensor_tensor_reduce (square+sum) on db; finale relu(s*(d1-d2)+c)
  ACT  : square-accumulate da -> d1a columns (in-place squares)

DMA completion ordering: completions of concurrently-issued DMAs are not
ordered, so each DMA gets a lane semaphore (u % NBUF); a lane's next DMA is
issue-gated on the consumer having retired that lane's previous supertile,
making per-lane cumulative counts race-free.  Same-engine dependent ops carry
self-waits (engine pipelining does not order them).  Semaphores are cleared
at the END of the program, leaving them zero for re-execution of this NEFF.
"""

import contextlib

import numpy as np
import ml_dtypes

try:
    import concourse.bass as bass
except ImportError:
    import sys

    sys.path.insert(0, "/opt/trn_rl_repo")
    import concourse.bass as bass

import concourse.mybir as mybir
from concourse.bass_utils import run_bass_kernel_spmd

B, D = 16384, 2048
NCORES = 8
SHARD = B // NCORES  # 2048 anchors per core
P = 128              # SBUF partitions
NT = SHARD // P      # 16 logical tiles per core
NST = NT // 2        # 8 supertiles (2 tiles per DRAM row)
MU = np.float32(136.72353790613718)
SIGMA = np.float32(62.34640414043511)

F32 = mybir.dt.float32
BF16 = mybir.dt.bfloat16
SQ = mybir.ActivationFunctionType.Square
NBUF = 4   # supertile slots == DMA lanes
NDB = 3    # da/db slots

_nc_cache = None


def _split_inline_waits(nc, max_inline=1):
    """The staged walrus build rejects compute instructions carrying more than
    one inline sync wait ("Too many sync wait commands").  Hoist excess waits
    onto standalone EventSemaphore instructions on the same engine, which is
    semantically identical (engine program order)."""
    cnt = 0
    for blk in nc.m.functions[0].blocks:
        new_insts = []
        for inst in blk.instructions:
            si = inst.sync_info
            if si is not None and len(si.on_wait) > max_inline:
                waits = list(si.on_wait)
                keep, hoist = waits[:max_inline], waits[max_inline:]
                for w in hoist:
                    cnt += 1
                    new_insts.append(
                        mybir.InstEventSemaphore(
                            name=f"{inst.name}-hoistw{cnt}",
                            engine=inst.engine,
                            sync_info=mybir.SyncInfo(on_wait=[w], on_update=[]),
                        )
                    )
                inst.sync_info = mybir.SyncInfo(
                    on_wait=keep, on_update=list(si.on_update)
                )
            new_insts.append(inst)
        blk.instructions = new_insts
    return nc


def _strip_init_barrier(nc):
    """Bass.__init__ unconditionally memsets 4 const APs and runs an
    all-engine barrier (~3.3us on HW) before the kernel body.  This kernel
    reads none of those const APs, so drop the memsets and that first barrier
    (everything up to the first branch in the entry block)."""
    blk = nc.m.functions[0].blocks[0]
    kept = []
    seen_branch = False
    for inst in blk.instructions:
        if isinstance(inst, mybir.InstUnconditionalBranch):
            seen_branch = True
        if not seen_branch:
            # Only const-AP memsets and the init barrier live before the
            # first branch; the kernel's own memset is inside a body block.
            if isinstance(
                inst, (mybir.InstMemset, mybir.InstDrain, mybir.InstEventSemaphore)
            ):
                continue
        kept.append(inst)
    blk.instructions = kept
    return nc


def build_nc():
    nc = bass.Bass(trn_type="TRN2")
    # one supertile row = [A|Ga|Gb] of anchor (2u*128+p) ++ same for anchor
    # ((2u+1)*128+p): 12288 bf16 = 24 KiB
    bundle = nc.declare_dram_parameter(
        "bundle", [NST * P, 6 * D], BF16, isOutput=False
    )
    sc = nc.declare_dram_parameter("sc", [P, 2 * NT], F32, isOutput=False)
    out = nc.declare_dram_parameter("out", [P, NT], F32, isOutput=True)

    b_r = bundle.rearrange("(u p) d -> u p d", p=P)  # [NST, P, 6D]

    es = contextlib.ExitStack()
    with es:
        T = [
            es.enter_context(nc.sbuf_tensor(f"Tb{i}", [P, 6 * D], BF16))
            for i in range(NBUF)
        ]
        da = [
            es.enter_context(nc.sbuf_tensor(f"da{i}", [P, 2 * D], BF16))
            for i in range(NDB)
        ]
        db = [
            es.enter_context(nc.sbuf_tensor(f"db{i}", [P, 2 * D], BF16))
            for i in range(NDB)
        ]
        sct = es.enter_context(nc.sbuf_tensor("sct", [P, 2 * NT], F32))
        d1a = es.enter_context(nc.sbuf_tensor("d1a", [P, NT], F32))
        d2a = es.enter_context(nc.sbuf_tensor("d2a", [P, NT], F32))
        dd = es.enter_context(nc.sbuf_tensor("dd", [P, NT], F32))
        pre = es.enter_context(nc.sbuf_tensor("pre", [P, NT], F32))
        loss = es.enter_context(nc.sbuf_tensor("loss", [P, NT], F32))
        zb = es.enter_context(nc.sbuf_tensor("zb", [P, 1], F32))  # ACT zero bias

        dlane = [es.enter_context(nc.semaphore(f"dl{i}")) for i in range(NBUF)]
        scsem = es.enter_context(nc.semaphore("scsem"))
        osem = es.enter_context(nc.semaphore("osem"))
        vsem = es.enter_context(nc.semaphore("vsem"))
        asem = es.enter_context(nc.semaphore("asem"))
        gsem = es.enter_context(nc.semaphore("gsem"))
        all_sems = dlane + [scsem, osem, vsem, asem, gsem]

        s_ap = sct[:, 0:NT]
        c_ap = sct[:, NT : 2 * NT]
        VTOT = 5 * NST  # DVE ops in the supertile loop
        ATOT = 3 * NST  # ACT ops in the supertile loop

        with nc.Block() as block:

            @block.gpsimd
            def _(gpsimd):
                gpsimd.memset(zb[:], 0.0).then_inc(gsem, 1)

            @block.sync
            def _(sync):
                for u in range(NST):
                    lane = dlane[u % NBUF]
                    if u >= NBUF:
                        # T slot free once the 4 subs of supertile u-NBUF
                        # retired; also orders this lane's previous DMA.
                        sync.wait_ge(vsem, 5 * (u - NBUF) + 4)
                        sync.wait_ge(lane, 16 * (u // NBUF))
                    sync.dma_start(out=T[u % NBUF][:], in_=b_r[u]).then_inc(lane, 16)
                    if u == 0:
                        sync.dma_start(out=sct[:], in_=sc[:]).then_inc(scsem, 16)
                sync.wait_ge(vsem, VTOT + 4)
                sync.dma_start(out=out[:], in_=loss[:]).then_inc(osem, 16)
                sync.wait_ge(osem, 16)

            @block.vector
            def _(vector):
                for u in range(NST):
                    A, Dq, Db = T[u % NBUF], da[u % NDB], db[u % NDB]
                    vector.wait_ge(dlane[u % NBUF], 16 * (u // NBUF + 1))
                    if u >= NDB:
                        # da/db slots free once ACT consumed supertile u-NDB
                        # (3 acts) and own square of u-NDB retired.
                        vector.wait_ge(asem, 3 * (u - NDB) + 3)
                        vector.wait_ge(vsem, 5 * (u - NDB) + 5)
                    nc.vector.tensor_sub(
                        Dq[:, 0:D], A[:, 0:D], A[:, D : 2 * D]
                    ).then_inc(vsem, 1)
                    nc.vector.tensor_sub(
                        Db[:, 0:D], A[:, 0:D], A[:, 2 * D : 3 * D]
                    ).then_inc(vsem, 1)
                    nc.vector.tensor_sub(
                        Dq[:, D : 2 * D], A[:, 3 * D : 4 * D], A[:, 4 * D : 5 * D]
                    ).then_inc(vsem, 1)
                    nc.vector.tensor_sub(
                        Db[:, D : 2 * D], A[:, 3 * D : 4 * D], A[:, 5 * D : 6 * D]
                    ).then_inc(vsem, 1)
                    # d2 col 0 via fused square+sum on DVE (in-place on db);
                    # needs own sub_b0 retired first (same-engine RAW).  ACT
                    # squares the other three columns (engine balance).
                    vector.wait_ge(vsem, 5 * u + 2)
                    nc.vector.scalar_tensor_tensor(
                        Db[:, 0:D], Db[:, 0:D], 1.0, Db[:, 0:D],
                        mybir.AluOpType.bypass, mybir.AluOpType.mult,
                        accum_out=d2a[:, 2 * u : 2 * u + 1],
                    ).then_inc(vsem, 1)
                vector.wait_ge(asem, ATOT)
                vector.wait_ge(scsem, 16)
                # self-wait: dd reads d2a written by this engine's own ttrs
                vector.wait_ge(vsem, VTOT)
                nc.vector.tensor_sub(dd[:], d1a[:], d2a[:]).then_inc(vsem, 1)
                vector.wait_ge(vsem, VTOT + 1)
                nc.vector.tensor_mul(pre[:], dd[:], s_ap).then_inc(vsem, 1)
                vector.wait_ge(vsem, VTOT + 2)
                nc.vector.tensor_add(pre[:], pre[:], c_ap).then_inc(vsem, 1)
                vector.wait_ge(vsem, VTOT + 3)
                nc.vector.tensor_scalar_max(loss[:], pre[:], 0.0).then_inc(vsem, 1)

            @block.scalar
            def _(scalar):
                scalar.wait_ge(gsem, 1)
                for u in range(NST):
                    Dq, Db = da[u % NDB], db[u % NDB]
                    # Square in place: only the accumulator is consumed.
                    scalar.wait_ge(vsem, 5 * u + 1)
                    nc.scalar.activation(
                        Dq[:, 0:D], Dq[:, 0:D], SQ,
                        bias=zb[:], accum_out=d1a[:, 2 * u : 2 * u + 1],
                    ).then_inc(asem, 1)
                    scalar.wait_ge(vsem, 5 * u + 3)
                    nc.scalar.activation(
                        Dq[:, D : 2 * D], Dq[:, D : 2 * D], SQ,
                        bias=zb[:], accum_out=d1a[:, 2 * u + 1 : 2 * u + 2],
                    ).then_inc(asem, 1)
                    scalar.wait_ge(vsem, 5 * u + 4)
                    nc.scalar.activation(
                        Db[:, D : 2 * D], Db[:, D : 2 * D], SQ,
                        bias=zb[:], accum_out=d2a[:, 2 * u + 1 : 2 * u + 2],
                    ).then_inc(asem, 1)

        # After the Block-exit all-engine barrier: leave the sems cleared for
        # any re-execution of this NEFF.
        for s in all_sems:
            nc.sync.sem_clear(s)

    return _split_inline_waits(_strip_init_barrier(nc))


def _host_scalars(label, idx1, idx2):
    """Flattened-scan label math: triplet indices (a, b), sign s = -1 if
    cond else +1, and c = 0.5*alpha, exactly as the reference computes them
    (f32 ops in the same order)."""
    r = np.asarray(label, dtype=np.float32)
    i1 = np.asarray(idx1).astype(np.int64)
    i2 = np.asarray(idx2).astype(np.int64)
    i = np.arange(B, dtype=np.int64)
    a = (i + 1 + i1 % (B - 1)) % B
    b = (i + 1 + i2 % (B - 1)) % B
    b = np.where(b == a, (i + 1 + (i2 + 1) % (B - 1)) % B, b)

    n = ((r - MU) / SIGMA).astype(np.float32)
    l1 = np.where(a < i, n[a], r[a]).astype(np.float32)
    l2 = np.where(b < i, n[b], r[b]).astype(np.float32)
    cond = np.abs(r - l1) >= np.abs(r - l2)
    near_l = np.where(cond, l2, l1)
    far_l = np.where(cond, l1, l2)
    nl = ((near_l - MU) / SIGMA).astype(np.float32)
    fl = ((far_l - MU) / SIGMA).astype(np.float32)
    alpha = ((n - fl) * (n - fl) - (n - nl) * (n - nl)).astype(np.float32)
    c = (np.float32(0.5) * alpha).astype(np.float32)
    s = np.where(cond, np.float32(-1.0), np.float32(1.0)).astype(np.float32)
    return a, b, s, c


def _host_prep(minibatch_features, label, idx1, idx2):
    feats = np.asarray(minibatch_features, dtype=np.float32)
    a, b, s, c = _host_scalars(label, idx1, idx2)
    fb = feats.astype(ml_dtypes.bfloat16)

    in_maps = []
    for ci in range(NCORES):
        sl = slice(ci * SHARD, (ci + 1) * SHARD)
        flat = np.concatenate([fb[sl], fb[a[sl]], fb[b[sl]]], axis=1)  # [SHARD,3D]
        # pair consecutive tiles into supertile rows: [NST, 2, P, 3D] ->
        # [NST, P, 2, 3D] -> [NST*P, 6D]
        bund = np.ascontiguousarray(
            flat.reshape(NST, 2, P, 3 * D).transpose(0, 2, 1, 3)
        ).reshape(NST * P, 6 * D)
        scm = np.empty((P, 2 * NT), dtype=np.float32)
        scm[:, :NT] = s[sl].reshape(NT, P).T
        scm[:, NT:] = c[sl].reshape(NT, P).T
        in_maps.append({"bundle": bund, "sc": scm})
    return in_maps


def _run_device(in_maps, trace=False, **kwargs):
    global _nc_cache
    if _nc_cache is None:
        _nc_cache = build_nc()
    return run_bass_kernel_spmd(
        _nc_cache, in_maps, core_ids=list(range(NCORES)), trace=trace, **kwargs
    )


def kernel(minibatch_features, label, idx1, idx2):
    in_maps = _host_prep(minibatch_features, label, idx1, idx2)
    res = _run_device(in_maps)
    total = np.float64(0.0)
    for ci in range(NCORES):
        total += np.asarray(res.results[ci]["out"], dtype=np.float64).sum()
    return np.asarray(total, dtype=np.float32)
